# revision 2
# baseline (speedup 1.0000x reference)
"""Batched Procrustes-alignment loss on 8 Trainium2 NeuronCores (v5).

v2 -> v3: merged p/t load pipeline (one PRB tile, shared mean tree/center),
Jacobi off-diagonal folding (exploits the exact zero at the previous
rotation's pivot -- no apq memsets, 2-op off-diag updates after rotation 1),
closed-form V after rotation 1 (skips the first V update), and wide [3*JF]
distance ops (6 DVE ops + 1 Act square for the whole rotate-subtract-square).
"""
import numpy as np
import ml_dtypes
import concourse.bass as bass
import concourse.mybir as mybir
import concourse.tile as tile
from concourse import bacc
from concourse.bass_utils import run_bass_kernel_spmd

AF = mybir.ActivationFunctionType
OP = mybir.AluOpType
AX = mybir.AxisListType
f32 = mybir.dt.float32
bf16 = mybir.dt.bfloat16

B, J, C = 262144, 17, 3
JC = J * C
NCORES = 8
BC = B // NCORES            # 32768
P = 128
F = 256
CHUNK = P * F               # 32768
NCHUNK = BC // CHUNK        # 1
SUB = 16
NSUB = F // SUB
ROT = 3
EPS = 1e-8
TINY = 1e-30
JF = J * F


def _ap(t, off, dims):
    a = t[:]
    return bass.AP(a.tensor, a.offset + off, [a.ap[0]] + dims)


def _pl(t, off, n):
    return _ap(t, off, [[1, n]])


def _bc(t, off, count, n=F):
    return _ap(t, off, [[0, count], [1, n]])


DEF_ENG = {
    "cvt": "scalar", "meantree": "vector", "meanscale": "scalar",
    "center": "vector",
    "sq": "scalar", "csum": "vector", "sqrtn": "scalar", "jsum": "vector",
    "sthin": "vector",
    "hprod": "vector", "htree": "vector", "hfin": "gpsimd", "hbf": "scalar",
    "amul": "gpsimd", "aadd": "gpsimd",
    "jthin": "vector", "jact": "scalar", "joff": "gpsimd", "jdiag": "gpsimd",
    "vupd": "vector", "csbf": "scalar", "vinit": "vector",
    "srtthin": "vector", "srtcopy": "gpsimd", "srtv": "vector",
    "sgn": "vector", "rsig": "vector", "rsact": "scalar",
    "uassm": "vector", "cross": "vector", "su2": "gpsimd", "rassm": "vector",
    "dmul": "vector", "dadd": "vector", "dsq": "scalar",
    "dcsum": "vector", "dsqrt": "scalar", "djtree": "vector", "dacc": "gpsimd",
}


def build_nc(engines=None, iters=1, rot=ROT, ablate=()):
    eng = dict(DEF_ENG)
    if engines:
        eng.update(engines)

    nc = bacc.Bacc("TRN2", target_bir_lowering=False)
    pred_d = nc.dram_tensor("pred", [BC, JC], f32, kind="ExternalInput")
    targ_d = nc.dram_tensor("target", [BC, JC], f32, kind="ExternalInput")
    signs_d = nc.dram_tensor("signs", [P, 4 * F * NCHUNK], bf16,
                             kind="ExternalInput")
    out_d = nc.dram_tensor("partial", [P, 1], f32, kind="ExternalOutput")

    def E(name):
        return getattr(nc, eng[name])

    with tile.TileContext(nc) as tc:
        with (
            tc.tile_pool(name="persist", bufs=1) as persist,
            tc.tile_pool(name="raw", bufs=2) as rawp,
            tc.tile_pool(name="pr", bufs=1) as prp,
            tc.tile_pool(name="grp", bufs=1) as grpp,
            tc.tile_pool(name="q17", bufs=4) as qp,
            tc.tile_pool(name="tr8", bufs=3) as trp,
            tc.tile_pool(name="y3", bufs=6) as y3p,
            tc.tile_pool(name="thin", bufs=8) as thinp,
        ):
            signs = persist.tile([P, 4 * F * NCHUNK], bf16, tag="signs",
                                 name="signs")
            nc.sync.dma_start(signs[:], signs_d[:])
            acc = persist.tile([P, F], f32, tag="acc", name="acc")
            nc.gpsimd.memset(acc[:], 0.0)

            def thin():
                return thinp.tile([P, F], f32, tag="thin", name="thin")

            def tbf():
                return thinp.tile([P, F], bf16, tag="cbf", name="cbf",
                                  bufs=4)

            def y3(dt=bf16):
                return y3p.tile([P, 3 * F], dt, tag="pr3", name="pr3",
                                bufs=3)

            def load_all(k):
                """DMA+convert chunk k of both tensors into one PRB
                [P, 6*JF] bf16 (t-major, then c-major planes), center both,
                and compute the two norm j-sums."""
                PRB = prp.tile([P, 6 * JF], bf16, tag="prb", name="prb")
                for ti, dram in ((0, pred_d), (1, targ_d)):
                    for s_ in range(NSUB):
                        raw = rawp.tile([P, JC * SUB], f32, tag=f"raw{ti}",
                                        name=f"raw{ti}")
                        nc.sync.dma_start(
                            raw[:], bass.AP(dram[:].tensor,
                                            (k * CHUNK + s_ * SUB) * JC,
                                            [[F * JC, P], [1, JC * SUB]]))
                        E("cvt").activation(
                            _ap(PRB, ti * 3 * JF + s_ * SUB,
                                [[JF, 3], [F, J], [1, SUB]]),
                            _ap(raw, 0, [[1, 3], [3, J], [JC, SUB]]), AF.Copy)
                # mean tree over j, per (tensor, c) plane via tr8
                meanB = grpp.tile([P, 6 * F], bf16, tag="mn", name="mn")
                for ti in range(2):
                    for c in range(3):
                        o = ti * 3 * JF + c * JF
                        T = trp.tile([P, 8 * F], bf16, tag="tr8", name="tr8")
                        E("meantree").tensor_tensor(
                            _pl(T, 0, 8 * F), _pl(PRB, o, 8 * F),
                            _pl(PRB, o + 8 * F, 8 * F), OP.add)
                        for hw in (4, 2, 1):
                            E("meantree").tensor_tensor(
                                _pl(T, 0, hw * F), _pl(T, 0, hw * F),
                                _pl(T, hw * F, hw * F), OP.add)
                        E("meantree").tensor_tensor(
                            _pl(T, 0, F), _pl(T, 0, F),
                            _pl(PRB, o + 16 * F, F), OP.add)
                        E("meanscale").activation(
                            _pl(meanB, (ti * 3 + c) * F, F), _pl(T, 0, F),
                            AF.Copy, scale=1.0 / J)
                # center in place (per tensor: one [3*JF] op)
                for ti in range(2):
                    E("center").tensor_tensor(
                        _pl(PRB, ti * 3 * JF, 3 * JF),
                        _pl(PRB, ti * 3 * JF, 3 * JF),
                        _ap(meanB, ti * 3 * F, [[F, 3], [0, J], [1, F]]),
                        OP.subtract)
                # per-joint norms -> j-sums (per tensor)
                pns = []
                for ti in range(2):
                    o = ti * 3 * JF
                    sq0 = qp.tile([P, JF], bf16, tag="q", name="sq")
                    sq1 = qp.tile([P, JF], bf16, tag="q", name="sq")
                    E("sq").activation(sq0[:], _pl(PRB, o, JF), AF.Square)
                    E("sq").activation(sq1[:], _pl(PRB, o + JF, JF),
                                       AF.Square)
                    E("csum").tensor_tensor(sq0[:], sq0[:], sq1[:], OP.add)
                    E("sq").activation(sq1[:], _pl(PRB, o + 2 * JF, JF),
                                       AF.Square)
                    E("csum").tensor_tensor(sq0[:], sq0[:], sq1[:], OP.add)
                    E("sqrtn").activation(sq0[:], sq0[:], AF.Sqrt)
                    T8 = trp.tile([P, 8 * F], bf16, tag="tr8", name="tr8")
                    E("jsum").tensor_tensor(
                        _pl(T8, 0, 8 * F), _pl(sq0, 0, 8 * F),
                        _pl(sq0, 8 * F, 8 * F), OP.add)
                    for hw in (4, 2, 1):
                        E("jsum").tensor_tensor(
                            _pl(T8, 0, hw * F), _pl(T8, 0, hw * F),
                            _pl(T8, hw * F, hw * F), OP.add)
                    pn = thin()
                    E("jsum").tensor_tensor(
                        pn[:], _pl(T8, 0, F), _pl(sq0, 16 * F, F), OP.add)
                    pns.append(pn)
                return PRB, pns[0], pns[1]

            def stage1(k):
                PRB, pn, tn = load_all(k)
                if "load" in ablate:
                    dj = thin()
                    E("sthin").tensor_tensor(dj[:], pn[:], tn[:], OP.add)
                    E("dacc").tensor_tensor(acc[:], acc[:], dj[:], OP.add)
                    return None
                pne = thin()
                E("sthin").tensor_scalar_add(pne[:], pn[:], EPS)
                rp = thin()
                nc.vector.reciprocal_approx_fast(rp[:], pne[:])
                s_sc = thin()
                E("sthin").tensor_tensor(s_sc[:], tn[:], rp[:], OP.mult)
                tne = thin()
                E("sthin").tensor_scalar_add(tne[:], tn[:], 1e-20)
                rt = thin()
                nc.vector.reciprocal_approx_fast(rt[:], tne[:])
                inv_s = thin()
                E("sthin").tensor_tensor(inv_s[:], pne[:], rt[:], OP.mult)

                # H_{r,kk} = sum_j PC_r TC_kk; H planes k-major: idx kk*3+r
                # 9 independent (i,kk) product+tree pipelines
                H = grpp.tile([P, 9 * F], f32, tag="H", name="H")
                for i in range(3):
                    for kk in range(3):
                        prod = qp.tile([P, JF], bf16, tag="q", name="hp")
                        E("hprod").tensor_tensor(
                            prod[:], _pl(PRB, i * JF, JF),
                            _pl(PRB, (3 + kk) * JF, JF), OP.mult)
                        T8 = trp.tile([P, 8 * F], bf16, tag="tr8",
                                      name="tr8")
                        E("htree").tensor_tensor(
                            _pl(T8, 0, 8 * F), _pl(prod, 0, 8 * F),
                            _pl(prod, 8 * F, 8 * F), OP.add)
                        for hw in (4, 2, 1):
                            E("htree").tensor_tensor(
                                _pl(T8, 0, hw * F), _pl(T8, 0, hw * F),
                                _pl(T8, hw * F, hw * F), OP.add)
                        E("hfin").tensor_tensor(
                            _pl(H, (kk * 3 + i) * F, F), _pl(T8, 0, F),
                            _pl(prod, 16 * F, F), OP.add)
                Hbf = grpp.tile([P, 9 * F], bf16, tag="Hbf", name="Hbf")
                E("hbf").activation(Hbf[:], H[:], AF.Copy)

                A_idx = {(0, 0): 0, (0, 1): 1, (0, 2): 2, (1, 1): 3,
                         (1, 2): 4, (2, 2): 5}
                A = grpp.tile([P, 6 * F], f32, tag="A", name="A")
                for (a, b), sl in A_idx.items():
                    pr3 = y3(f32)
                    E("amul").tensor_tensor(
                        pr3[:], _pl(H, a * 3 * F, 3 * F),
                        _pl(H, b * 3 * F, 3 * F), OP.mult)
                    t_ = thin()
                    E("aadd").tensor_tensor(
                        t_[:], _pl(pr3, 0, F), _pl(pr3, F, F), OP.add)
                    E("aadd").tensor_tensor(
                        _pl(A, sl * F, F), t_[:], _pl(pr3, 2 * F, F), OP.add)
                if "stage1" in ablate:
                    dj = thin()
                    E("sthin").tensor_tensor(dj[:], _pl(A, 0, F), s_sc[:],
                                             OP.add)
                    E("dacc").tensor_tensor(acc[:], acc[:], dj[:], OP.add)
                    return None
                return dict(PRB=PRB, H=H, Hbf=Hbf, A=A, A_idx=A_idx,
                            s_sc=s_sc, inv_s=inv_s)

            def stage2(k, st):
                if st is None:
                    return
                PRB, H, Hbf = st["PRB"], st["H"], st["Hbf"]
                A, A_idx = st["A"], st["A_idx"]
                s_sc, inv_s = st["s_sc"], st["inv_s"]

                def Ap(a, b):
                    return _pl(A, A_idx[(min(a, b), max(a, b))] * F, F)

                if "nosvd" in ablate:
                    dist_phase(k, PRB, Hbf)
                    return

                V = grpp.tile([P, 9 * F], bf16, tag="V", name="V")

                def VC(i):
                    return _ap(V, i * 3 * F, [[F, 3], [1, F]])

                def Vb(kk, i):
                    return _ap(V, (i * 3 + kk) * F, [[0, 3], [1, F]])

                order = [(0, 1, 2), (0, 2, 1), (1, 2, 0)] * 3
                prev_pq = None
                for rn, (p_, q_, r_) in enumerate(order[:rot]):
                    app, aqq, apq = Ap(p_, p_), Ap(q_, q_), Ap(p_, q_)
                    tau = thin()
                    E("jthin").tensor_tensor(tau[:], aqq, app, OP.subtract)
                    u = thin()
                    E("jthin").tensor_tensor(u[:], tau[:], tau[:], OP.mult)
                    ap2 = thin()
                    E("jthin").tensor_tensor(ap2[:], apq, apq, OP.mult)
                    z = thin()
                    E("jthin").scalar_tensor_tensor(
                        z[:], ap2[:], 4.0, u[:], OP.mult, OP.add)
                    y = thin()
                    E("jthin").tensor_tensor(y[:], u[:], z[:], OP.mult)
                    w = thin()
                    E("jact").activation(w[:], y[:], AF.Sqrt)
                    den = thin()
                    E("jthin").scalar_tensor_tensor(
                        den[:], w[:], TINY, u[:], OP.add, OP.add)
                    rden = thin()
                    nc.vector.reciprocal_approx_fast(rden[:], den[:])
                    num = thin()
                    E("jthin").scalar_tensor_tensor(
                        num[:], apq, 2.0, tau[:], OP.mult, OP.mult)
                    t_ = thin()
                    E("jthin").tensor_tensor(t_[:], num[:], rden[:], OP.mult)
                    tsq = thin()
                    E("jthin").tensor_tensor(tsq[:], t_[:], t_[:], OP.mult)
                    sv = thin()
                    E("jact").activation(sv[:], tsq[:], AF.Sqrt, bias=1.0)
                    c_ = thin()
                    nc.vector.reciprocal_approx_fast(c_[:], sv[:])
                    s_ = thin()
                    E("jthin").tensor_tensor(s_[:], t_[:], c_[:], OP.mult)
                    tap = thin()
                    E("jdiag").tensor_tensor(tap[:], t_[:], apq, OP.mult)
                    E("jdiag").tensor_tensor(app, app, tap[:], OP.subtract)
                    E("jdiag").tensor_tensor(aqq, aqq, tap[:], OP.add)
                    # off-diagonal update: the slot zeroed by the previous
                    # rotation is exactly zero -> 2-op folded form
                    rp_s = (min(r_, p_), max(r_, p_))
                    rq_s = (min(r_, q_), max(r_, q_))
                    arp, arq = Ap(r_, p_), Ap(r_, q_)
                    if prev_pq is None:
                        x1, x2, x3, x4 = thin(), thin(), thin(), thin()
                        E("joff").tensor_tensor(x1[:], arp, c_[:], OP.mult)
                        E("joff").tensor_tensor(x2[:], arq, s_[:], OP.mult)
                        E("joff").tensor_tensor(x3[:], arp, s_[:], OP.mult)
                        E("joff").tensor_tensor(x4[:], arq, c_[:], OP.mult)
                        E("joff").tensor_tensor(arp, x1[:], x2[:], OP.subtract)
                        E("joff").tensor_tensor(arq, x3[:], x4[:], OP.add)
                    elif prev_pq == rp_s:
                        # arp == 0: new arp = -arq*s ; new arq = arq*c
                        E("jthin").scalar_tensor_tensor(
                            arp, arq, -1.0, s_[:], OP.mult, OP.mult)
                        E("joff").tensor_tensor(arq, arq, c_[:], OP.mult)
                    else:
                        assert prev_pq == rq_s
                        # arq == 0: new arq = arp*s ; new arp = arp*c
                        E("joff").tensor_tensor(arq, arp, s_[:], OP.mult)
                        E("joff").tensor_tensor(arp, arp, c_[:], OP.mult)
                    prev_pq = (min(p_, q_), max(p_, q_))
                    cbf = tbf()
                    sbf = tbf()
                    E("csbf").activation(cbf[:], c_[:], AF.Copy)
                    E("csbf").activation(sbf[:], s_[:], AF.Copy)
                    if rn == 0:
                        # V = G1 directly (closed form, V was identity)
                        nc.gpsimd.memset(V[:], 0.0)
                        E("vinit").tensor_copy(_pl(V, (p_ * 3 + p_) * F, F),
                                               cbf[:])
                        E("vinit").tensor_scalar_mul(
                            _pl(V, (p_ * 3 + q_) * F, F), sbf[:], -1.0)
                        E("vinit").tensor_copy(_pl(V, (q_ * 3 + p_) * F, F),
                                               sbf[:])
                        E("vinit").tensor_copy(_pl(V, (q_ * 3 + q_) * F, F),
                                               cbf[:])
                        nc.gpsimd.memset(_pl(V, (r_ * 3 + r_) * F, F), 1.0)
                    else:
                        y1, y2, y3_, y4 = y3(), y3(), y3(), y3()
                        E("vupd").tensor_tensor(y1[:], VC(p_),
                                                _bc(cbf, 0, 3), OP.mult)
                        E("vupd").tensor_tensor(y2[:], VC(q_),
                                                _bc(sbf, 0, 3), OP.mult)
                        E("vupd").tensor_tensor(y3_[:], VC(p_),
                                                _bc(sbf, 0, 3), OP.mult)
                        E("vupd").tensor_tensor(y4[:], VC(q_),
                                                _bc(cbf, 0, 3), OP.mult)
                        E("vupd").tensor_tensor(VC(p_), y1[:], y2[:],
                                                OP.subtract)
                        E("vupd").tensor_tensor(VC(q_), y3_[:], y4[:],
                                                OP.add)

                # ---- sort + detV parity
                lam = [Ap(0, 0), Ap(1, 1), Ap(2, 2)]
                detV = thin()
                first = True
                for (i, j) in ((0, 1), (0, 2), (1, 2)):
                    m = thin()
                    E("srtthin").tensor_tensor(m[:], lam[j], lam[i], OP.is_gt)
                    lo = thin()
                    E("srtthin").tensor_tensor(lo[:], lam[i], lam[j], OP.min)
                    E("srtthin").tensor_tensor(lam[i], lam[i], lam[j], OP.max)
                    E("srtcopy").tensor_copy(lam[j], lo[:])
                    mbf = tbf()
                    E("csbf").activation(mbf[:], m[:], AF.Copy)
                    d3 = y3()
                    md = y3()
                    E("srtv").tensor_tensor(d3[:], VC(j), VC(i), OP.subtract)
                    E("srtv").tensor_tensor(md[:], d3[:], _bc(mbf, 0, 3),
                                            OP.mult)
                    E("srtv").tensor_tensor(VC(i), VC(i), md[:], OP.add)
                    E("srtv").tensor_tensor(VC(j), VC(j), md[:], OP.subtract)
                    if first:
                        nc.vector.tensor_scalar(detV[:], m[:], -2.0, 1.0,
                                                OP.mult, OP.add)
                        first = False
                    else:
                        f_ = thin()
                        nc.vector.tensor_scalar(f_[:], m[:], -2.0, 1.0,
                                                OP.mult, OP.add)
                        E("srtthin").tensor_tensor(detV[:], detV[:], f_[:],
                                                   OP.mult)

                # ---- random sign flips on V columns (one wide op)
                soff = 4 * F * k
                E("sgn").tensor_tensor(
                    V[:], V[:],
                    _ap(signs, soff, [[F, 3], [0, 3], [1, F]]), OP.mult)

                # ---- U
                U = grpp.tile([P, 9 * F], bf16, tag="U", name="U")

                def UC(i):
                    return _ap(U, i * 3 * F, [[F, 3], [1, F]])

                def Up(r, i):
                    return _pl(U, (i * 3 + r) * F, F)

                for i in range(2):
                    rl = thin()
                    E("rsact").activation(rl[:], lam[i], AF.Relu)
                    sg_ = thin()
                    E("rsact").activation(sg_[:], rl[:], AF.Sqrt)
                    sge = thin()
                    E("rsig").tensor_scalar_add(sge[:], sg_[:], 1e-20)
                    rs = thin()
                    nc.vector.reciprocal_approx_fast(rs[:], sge[:])
                    E("rsig").tensor_tensor(rs[:], rs[:], s_sc[:], OP.mult)
                    rsbf = tbf()
                    E("csbf").activation(rsbf[:], rs[:], AF.Copy)
                    wv = y3()
                    w2 = y3()
                    E("uassm").tensor_tensor(
                        wv[:], _pl(Hbf, 0, 3 * F),
                        _ap(V, (i * 3 + 0) * F, [[0, 3], [1, F]]), OP.mult)
                    E("uassm").tensor_tensor(
                        w2[:], _pl(Hbf, 3 * F, 3 * F),
                        _ap(V, (i * 3 + 1) * F, [[0, 3], [1, F]]), OP.mult)
                    E("uassm").tensor_tensor(wv[:], wv[:], w2[:], OP.add)
                    E("uassm").tensor_tensor(
                        w2[:], _pl(Hbf, 6 * F, 3 * F),
                        _ap(V, (i * 3 + 2) * F, [[0, 3], [1, F]]), OP.mult)
                    E("uassm").tensor_tensor(wv[:], wv[:], w2[:], OP.add)
                    E("uassm").tensor_tensor(UC(i), wv[:], _bc(rsbf, 0, 3),
                                             OP.mult)
                cr = [(1, 2), (2, 0), (0, 1)]
                for r in range(3):
                    a1, a2 = cr[r]
                    t1 = tbf()
                    t2 = tbf()
                    E("cross").tensor_tensor(t1[:], Up(a1, 0), Up(a2, 1),
                                             OP.mult)
                    E("cross").tensor_tensor(t2[:], Up(a2, 0), Up(a1, 1),
                                             OP.mult)
                    E("cross").tensor_tensor(Up(r, 2), t1[:], t2[:],
                                             OP.subtract)
                su2 = thin()
                E("su2").tensor_tensor(su2[:], detV[:],
                                       _pl(signs, soff + 3 * F, F), OP.mult)
                E("su2").tensor_tensor(su2[:], su2[:], inv_s[:], OP.mult)
                su2bf = tbf()
                E("csbf").activation(su2bf[:], su2[:], AF.Copy)
                E("su2").tensor_tensor(UC(2), UC(2), _bc(su2bf, 0, 3),
                                       OP.mult)

                # ---- R'' assembly (wide): R[a][b] = sum_i U[b,i] V[i,a]
                R = grpp.tile([P, 9 * F], bf16, tag="R", name="R")
                for a in range(3):
                    p1 = y3()
                    p2 = y3()
                    E("rassm").tensor_tensor(p1[:], UC(0), Vb(0, a), OP.mult)
                    E("rassm").tensor_tensor(p2[:], UC(1), Vb(1, a), OP.mult)
                    E("rassm").tensor_tensor(p1[:], p1[:], p2[:], OP.add)
                    E("rassm").tensor_tensor(p2[:], UC(2), Vb(2, a), OP.mult)
                    E("rassm").tensor_tensor(
                        _ap(R, a * 3 * F, [[F, 3], [1, F]]),
                        p1[:], p2[:], OP.add)
                if "nodist" in ablate:
                    dj = thin()
                    E("sthin").tensor_tensor(dj[:], _pl(R, 0, F), detV[:],
                                             OP.add)
                    E("dacc").tensor_tensor(acc[:], acc[:], dj[:], OP.add)
                    return
                dist_phase(k, PRB, R)

            def dist_phase(k, PRB, R):
                def Rb(a, b):
                    return _bc(R, (a * 3 + b) * F, J)

                d2t = qp.tile([P, JF], bf16, tag="q", name="d2t")
                for c in range(3):
                    q = qp.tile([P, JF], bf16, tag="q", name="q")
                    t2_ = qp.tile([P, JF], bf16, tag="q", name="q")
                    E("dmul").tensor_tensor(q[:], _pl(PRB, 0, JF), Rb(c, 0),
                                            OP.mult)
                    E("dmul").tensor_tensor(t2_[:], _pl(PRB, JF, JF),
                                            Rb(c, 1), OP.mult)
                    E("dadd").tensor_tensor(q[:], q[:], t2_[:], OP.add)
                    E("dmul").tensor_tensor(t2_[:], _pl(PRB, 2 * JF, JF),
                                            Rb(c, 2), OP.mult)
                    E("dadd").tensor_tensor(q[:], q[:], t2_[:], OP.add)
                    E("dadd").tensor_tensor(q[:], q[:],
                                            _pl(PRB, (3 + c) * JF, JF),
                                            OP.subtract)
                    if c == 0:
                        E("dsq").activation(d2t[:], q[:], AF.Square)
                    else:
                        E("dsq").activation(q[:], q[:], AF.Square)
                        E("dcsum").tensor_tensor(d2t[:], d2t[:], q[:], OP.add)
                E("dsqrt").activation(d2t[:], d2t[:], AF.Sqrt)
                T8 = trp.tile([P, 8 * F], bf16, tag="tr8", name="tr8")
                E("djtree").tensor_tensor(
                    _pl(T8, 0, 8 * F), _pl(d2t, 0, 8 * F),
                    _pl(d2t, 8 * F, 8 * F), OP.add)
                for hw in (4, 2, 1):
                    E("djtree").tensor_tensor(
                        _pl(T8, 0, hw * F), _pl(T8, 0, hw * F),
                        _pl(T8, hw * F, hw * F), OP.add)
                dj = thin()
                E("djtree").tensor_tensor(dj[:], _pl(T8, 0, F),
                                          _pl(d2t, 16 * F, F), OP.add)
                E("dacc").tensor_tensor(acc[:], acc[:], dj[:], OP.add)

            def whole_body():
                sts = [stage1(k) for k in range(NCHUNK)]
                for k in range(NCHUNK):
                    stage2(k, sts[k])

            if iters == 1:
                whole_body()
            else:
                with tc.For_i(0, iters, 1):
                    whole_body()

            accs = persist.tile([P, 1], f32, tag="accs", name="accs")
            nc.vector.tensor_reduce(accs[:], acc[:], axis=AX.X, op=OP.add)
            nc.sync.dma_start(out_d[:], accs[:])

    nc.compile()
    return nc


_sign_planes = None


def sign_planes():
    global _sign_planes
    if _sign_planes is None:
        rng = np.random.default_rng(20260805)
        s = rng.choice(np.float32([-1.0, 1.0]), size=(3, P, F * NCHUNK))
        sp = np.empty((P, 4 * F * NCHUNK), np.float32)
        for k in range(NCHUNK):
            o = 4 * F * k
            for i in range(3):
                sp[:, o + i * F:o + (i + 1) * F] = s[i, :, k * F:(k + 1) * F]
            sp[:, o + 3 * F:o + 4 * F] = \
                (s[0] * s[1] * s[2])[:, k * F:(k + 1) * F]
        _sign_planes = sp.astype(ml_dtypes.bfloat16)
    return _sign_planes


_nc_cache = None


def get_nc():
    global _nc_cache
    if _nc_cache is None:
        _nc_cache = build_nc()
    return _nc_cache


def run(nc, pred, target, trace=False, **kw):
    pred2 = np.ascontiguousarray(np.asarray(pred), np.float32).reshape(B, JC)
    targ2 = np.ascontiguousarray(np.asarray(target), np.float32).reshape(B, JC)
    sp = sign_planes()
    in_maps = [
        {"pred": pred2[c * BC:(c + 1) * BC],
         "target": targ2[c * BC:(c + 1) * BC], "signs": sp}
        for c in range(NCORES)
    ]
    res = run_bass_kernel_spmd(nc, in_maps, list(range(NCORES)), trace=trace,
                               **kw)
    total = sum(r["partial"].astype(np.float64).sum() for r in res.results)
    loss = np.float32(total / (B * J))
    return loss, res


def kernel(pred, target):
    loss, _ = run(get_nc(), pred, target)
    return loss


# revision 3
# speedup vs baseline: 1.0328x; 1.0328x over previous
"""Batched Procrustes-alignment loss on 8 Trainium2 NeuronCores (v5).

v2 -> v3: merged p/t load pipeline (one PRB tile, shared mean tree/center),
Jacobi off-diagonal folding (exploits the exact zero at the previous
rotation's pivot -- no apq memsets, 2-op off-diag updates after rotation 1),
closed-form V after rotation 1 (skips the first V update), and wide [3*JF]
distance ops (6 DVE ops + 1 Act square for the whole rotate-subtract-square).
"""
import numpy as np
import ml_dtypes
import concourse.bass as bass
import concourse.mybir as mybir
import concourse.tile as tile
from concourse import bacc
from concourse.bass_utils import run_bass_kernel_spmd

AF = mybir.ActivationFunctionType
OP = mybir.AluOpType
AX = mybir.AxisListType
f32 = mybir.dt.float32
bf16 = mybir.dt.bfloat16

B, J, C = 262144, 17, 3
JC = J * C
NCORES = 8
BC = B // NCORES            # 32768
P = 128
F = 256
CHUNK = P * F               # 32768
NCHUNK = BC // CHUNK        # 1
SUB = 16
NSUB = F // SUB
ROT = 3
EPS = 1e-8
TINY = 1e-30
JF = J * F


def _ap(t, off, dims):
    a = t[:]
    return bass.AP(a.tensor, a.offset + off, [a.ap[0]] + dims)


def _pl(t, off, n):
    return _ap(t, off, [[1, n]])


def _bc(t, off, count, n=F):
    return _ap(t, off, [[0, count], [1, n]])


def _rows(t, off, nrows):
    """2-D view [[F, nrows], [1, F]] — keeps the DVE fast path (long 1-D
    packed APs fall off the 2x/4x modes)."""
    return _ap(t, off, [[F, nrows], [1, F]])


DEF_ENG = {
    "cvt": "scalar", "meantree": "vector", "meanscale": "scalar",
    "center": "vector",
    "sq": "scalar", "csum": "vector", "sqrtn": "scalar", "jsum": "vector",
    "sthin": "vector",
    "hprod": "vector", "htree": "vector", "hfin": "gpsimd", "hbf": "scalar",
    "amul": "gpsimd", "aadd": "gpsimd",
    "jthin": "vector", "jact": "scalar", "joff": "gpsimd", "jdiag": "gpsimd",
    "vupd": "vector", "csbf": "scalar", "vinit": "vector",
    "srtthin": "vector", "srtcopy": "gpsimd", "srtv": "vector",
    "sgn": "vector", "rsig": "vector", "rsact": "scalar",
    "uassm": "vector", "cross": "vector", "su2": "gpsimd", "rassm": "vector",
    "dmul": "vector", "dadd": "vector", "dsq": "scalar",
    "dcsum": "vector", "dsqrt": "scalar", "djtree": "vector", "dacc": "gpsimd",
}


def build_nc(engines=None, iters=1, rot=ROT, ablate=()):
    eng = dict(DEF_ENG)
    if engines:
        eng.update(engines)

    nc = bacc.Bacc("TRN2", target_bir_lowering=False)
    pred_d = nc.dram_tensor("pred", [BC, JC], f32, kind="ExternalInput")
    targ_d = nc.dram_tensor("target", [BC, JC], f32, kind="ExternalInput")
    signs_d = nc.dram_tensor("signs", [P, 4 * F * NCHUNK], bf16,
                             kind="ExternalInput")
    out_d = nc.dram_tensor("partial", [P, 1], f32, kind="ExternalOutput")

    def E(name):
        return getattr(nc, eng[name])

    with tile.TileContext(nc) as tc:
        with (
            tc.tile_pool(name="persist", bufs=1) as persist,
            tc.tile_pool(name="raw", bufs=2) as rawp,
            tc.tile_pool(name="pr", bufs=1) as prp,
            tc.tile_pool(name="grp", bufs=1) as grpp,
            tc.tile_pool(name="q17", bufs=4) as qp,
            tc.tile_pool(name="tr8", bufs=3) as trp,
            tc.tile_pool(name="y3", bufs=6) as y3p,
            tc.tile_pool(name="thin", bufs=8) as thinp,
        ):
            signs = persist.tile([P, 4 * F * NCHUNK], bf16, tag="signs",
                                 name="signs")
            nc.sync.dma_start(signs[:], signs_d[:])
            acc = persist.tile([P, F], f32, tag="acc", name="acc")
            nc.gpsimd.memset(acc[:], 0.0)

            def thin():
                return thinp.tile([P, F], f32, tag="thin", name="thin")

            def tbf():
                return thinp.tile([P, F], bf16, tag="cbf", name="cbf",
                                  bufs=4)

            def y3(dt=bf16):
                return y3p.tile([P, 3 * F], dt, tag="pr3", name="pr3",
                                bufs=3)

            def load_all(k):
                """DMA+convert chunk k of both tensors into one PRB
                [P, 6*JF] bf16 (t-major, then c-major planes), center both,
                and compute the two norm j-sums."""
                PRB = prp.tile([P, 6 * JF], bf16, tag="prb", name="prb")
                for ti, dram in ((0, pred_d), (1, targ_d)):
                    for s_ in range(NSUB):
                        raw = rawp.tile([P, JC * SUB], f32, tag=f"raw{ti}",
                                        name=f"raw{ti}")
                        nc.sync.dma_start(
                            raw[:], bass.AP(dram[:].tensor,
                                            (k * CHUNK + s_ * SUB) * JC,
                                            [[F * JC, P], [1, JC * SUB]]))
                        E("cvt").activation(
                            _ap(PRB, ti * 3 * JF + s_ * SUB,
                                [[JF, 3], [F, J], [1, SUB]]),
                            _ap(raw, 0, [[1, 3], [3, J], [JC, SUB]]), AF.Copy)
                # mean tree over j, per (tensor, c) plane via tr8
                meanB = grpp.tile([P, 6 * F], bf16, tag="mn", name="mn")
                for ti in range(2):
                    for c in range(3):
                        o = ti * 3 * JF + c * JF
                        T = trp.tile([P, 8 * F], bf16, tag="tr8", name="tr8")
                        E("meantree").tensor_tensor(
                            _rows(T, 0, 8), _rows(PRB, o, 8),
                            _rows(PRB, o + 8 * F, 8), OP.add)
                        for hw in (4, 2):
                            E("meantree").tensor_tensor(
                                _rows(T, 0, hw), _rows(T, 0, hw),
                                _rows(T, hw * F, hw), OP.add)
                        E("meantree").tensor_tensor(
                            _pl(T, 0, F), _pl(T, 0, F), _pl(T, F, F), OP.add)
                        E("meantree").tensor_tensor(
                            _pl(T, 0, F), _pl(T, 0, F),
                            _pl(PRB, o + 16 * F, F), OP.add)
                        E("meanscale").activation(
                            _pl(meanB, (ti * 3 + c) * F, F), _pl(T, 0, F),
                            AF.Copy, scale=1.0 / J)
                # center in place (per tensor: one [3*JF] op)
                for ti in range(2):
                    E("center").tensor_tensor(
                        _rows(PRB, ti * 3 * JF, 3 * J),
                        _rows(PRB, ti * 3 * JF, 3 * J),
                        _ap(meanB, ti * 3 * F, [[F, 3], [0, J], [1, F]]),
                        OP.subtract)
                # per-joint norms -> j-sums (per tensor)
                pns = []
                for ti in range(2):
                    o = ti * 3 * JF
                    sq0 = qp.tile([P, JF], bf16, tag="q", name="sq")
                    sq1 = qp.tile([P, JF], bf16, tag="q", name="sq")
                    E("sq").activation(_rows(sq0, 0, J), _rows(PRB, o, J),
                                       AF.Square)
                    E("sq").activation(_rows(sq1, 0, J),
                                       _rows(PRB, o + JF, J), AF.Square)
                    E("csum").tensor_tensor(_rows(sq0, 0, J), _rows(sq0, 0, J),
                                            _rows(sq1, 0, J), OP.add)
                    E("sq").activation(_rows(sq1, 0, J),
                                       _rows(PRB, o + 2 * JF, J), AF.Square)
                    E("csum").tensor_tensor(_rows(sq0, 0, J), _rows(sq0, 0, J),
                                            _rows(sq1, 0, J), OP.add)
                    E("sqrtn").activation(_rows(sq0, 0, J), _rows(sq0, 0, J),
                                          AF.Sqrt)
                    T8 = trp.tile([P, 8 * F], bf16, tag="tr8", name="tr8")
                    E("jsum").tensor_tensor(
                        _rows(T8, 0, 8), _rows(sq0, 0, 8),
                        _rows(sq0, 8 * F, 8), OP.add)
                    for hw in (4, 2):
                        E("jsum").tensor_tensor(
                            _rows(T8, 0, hw), _rows(T8, 0, hw),
                            _rows(T8, hw * F, hw), OP.add)
                    E("jsum").tensor_tensor(
                        _pl(T8, 0, F), _pl(T8, 0, F), _pl(T8, F, F), OP.add)
                    pn = thin()
                    E("jsum").tensor_tensor(
                        pn[:], _pl(T8, 0, F), _pl(sq0, 16 * F, F), OP.add)
                    pns.append(pn)
                return PRB, pns[0], pns[1]

            def stage1(k):
                PRB, pn, tn = load_all(k)
                if "load" in ablate:
                    dj = thin()
                    E("sthin").tensor_tensor(dj[:], pn[:], tn[:], OP.add)
                    E("dacc").tensor_tensor(acc[:], acc[:], dj[:], OP.add)
                    return None
                pne = thin()
                E("sthin").tensor_scalar_add(pne[:], pn[:], EPS)
                rp = thin()
                nc.vector.reciprocal_approx_fast(rp[:], pne[:])
                s_sc = thin()
                E("sthin").tensor_tensor(s_sc[:], tn[:], rp[:], OP.mult)
                tne = thin()
                E("sthin").tensor_scalar_add(tne[:], tn[:], 1e-20)
                rt = thin()
                nc.vector.reciprocal_approx_fast(rt[:], tne[:])
                inv_s = thin()
                E("sthin").tensor_tensor(inv_s[:], pne[:], rt[:], OP.mult)

                # H_{r,kk} = sum_j PC_r TC_kk; H planes k-major: idx kk*3+r
                # 9 independent (i,kk) product+tree pipelines
                H = grpp.tile([P, 9 * F], f32, tag="H", name="H")
                for i in range(3):
                    for kk in range(3):
                        prod = qp.tile([P, JF], bf16, tag="q", name="hp")
                        E("hprod").tensor_tensor(
                            _rows(prod, 0, J), _rows(PRB, i * JF, J),
                            _rows(PRB, (3 + kk) * JF, J), OP.mult)
                        T8 = trp.tile([P, 8 * F], bf16, tag="tr8",
                                      name="tr8")
                        E("htree").tensor_tensor(
                            _rows(T8, 0, 8), _rows(prod, 0, 8),
                            _rows(prod, 8 * F, 8), OP.add)
                        for hw in (4, 2):
                            E("htree").tensor_tensor(
                                _rows(T8, 0, hw), _rows(T8, 0, hw),
                                _rows(T8, hw * F, hw), OP.add)
                        E("htree").tensor_tensor(
                            _pl(T8, 0, F), _pl(T8, 0, F), _pl(T8, F, F),
                            OP.add)
                        E("hfin").tensor_tensor(
                            _pl(H, (kk * 3 + i) * F, F), _pl(T8, 0, F),
                            _pl(prod, 16 * F, F), OP.add)
                Hbf = grpp.tile([P, 9 * F], bf16, tag="Hbf", name="Hbf")
                E("hbf").activation(Hbf[:], H[:], AF.Copy)

                A_idx = {(0, 0): 0, (0, 1): 1, (0, 2): 2, (1, 1): 3,
                         (1, 2): 4, (2, 2): 5}
                A = grpp.tile([P, 6 * F], f32, tag="A", name="A")
                for (a, b), sl in A_idx.items():
                    pr3 = y3(f32)
                    E("amul").tensor_tensor(
                        pr3[:], _pl(H, a * 3 * F, 3 * F),
                        _pl(H, b * 3 * F, 3 * F), OP.mult)
                    t_ = thin()
                    E("aadd").tensor_tensor(
                        t_[:], _pl(pr3, 0, F), _pl(pr3, F, F), OP.add)
                    E("aadd").tensor_tensor(
                        _pl(A, sl * F, F), t_[:], _pl(pr3, 2 * F, F), OP.add)
                if "stage1" in ablate:
                    dj = thin()
                    E("sthin").tensor_tensor(dj[:], _pl(A, 0, F), s_sc[:],
                                             OP.add)
                    E("dacc").tensor_tensor(acc[:], acc[:], dj[:], OP.add)
                    return None
                return dict(PRB=PRB, H=H, Hbf=Hbf, A=A, A_idx=A_idx,
                            s_sc=s_sc, inv_s=inv_s)

            def stage2(k, st):
                if st is None:
                    return
                PRB, H, Hbf = st["PRB"], st["H"], st["Hbf"]
                A, A_idx = st["A"], st["A_idx"]
                s_sc, inv_s = st["s_sc"], st["inv_s"]

                def Ap(a, b):
                    return _pl(A, A_idx[(min(a, b), max(a, b))] * F, F)

                if "nosvd" in ablate:
                    dist_phase(k, PRB, Hbf)
                    return

                V = grpp.tile([P, 9 * F], bf16, tag="V", name="V")

                def VC(i):
                    return _ap(V, i * 3 * F, [[F, 3], [1, F]])

                def Vb(kk, i):
                    return _ap(V, (i * 3 + kk) * F, [[0, 3], [1, F]])

                order = [(0, 1, 2), (0, 2, 1), (1, 2, 0)] * 3
                prev_pq = None
                for rn, (p_, q_, r_) in enumerate(order[:rot]):
                    app, aqq, apq = Ap(p_, p_), Ap(q_, q_), Ap(p_, q_)
                    tau = thin()
                    E("jthin").tensor_tensor(tau[:], aqq, app, OP.subtract)
                    u = thin()
                    E("jthin").tensor_tensor(u[:], tau[:], tau[:], OP.mult)
                    ap2 = thin()
                    E("jthin").tensor_tensor(ap2[:], apq, apq, OP.mult)
                    z = thin()
                    E("jthin").scalar_tensor_tensor(
                        z[:], ap2[:], 4.0, u[:], OP.mult, OP.add)
                    y = thin()
                    E("jthin").tensor_tensor(y[:], u[:], z[:], OP.mult)
                    w = thin()
                    E("jact").activation(w[:], y[:], AF.Sqrt)
                    den = thin()
                    E("jthin").scalar_tensor_tensor(
                        den[:], w[:], TINY, u[:], OP.add, OP.add)
                    rden = thin()
                    nc.vector.reciprocal_approx_fast(rden[:], den[:])
                    num = thin()
                    E("jthin").scalar_tensor_tensor(
                        num[:], apq, 2.0, tau[:], OP.mult, OP.mult)
                    t_ = thin()
                    E("jthin").tensor_tensor(t_[:], num[:], rden[:], OP.mult)
                    tsq = thin()
                    E("jthin").tensor_tensor(tsq[:], t_[:], t_[:], OP.mult)
                    sv = thin()
                    E("jact").activation(sv[:], tsq[:], AF.Sqrt, bias=1.0)
                    c_ = thin()
                    nc.vector.reciprocal_approx_fast(c_[:], sv[:])
                    s_ = thin()
                    E("jthin").tensor_tensor(s_[:], t_[:], c_[:], OP.mult)
                    tap = thin()
                    E("jdiag").tensor_tensor(tap[:], t_[:], apq, OP.mult)
                    E("jdiag").tensor_tensor(app, app, tap[:], OP.subtract)
                    E("jdiag").tensor_tensor(aqq, aqq, tap[:], OP.add)
                    # off-diagonal update: the slot zeroed by the previous
                    # rotation is exactly zero -> 2-op folded form
                    rp_s = (min(r_, p_), max(r_, p_))
                    rq_s = (min(r_, q_), max(r_, q_))
                    arp, arq = Ap(r_, p_), Ap(r_, q_)
                    if prev_pq is None:
                        x1, x2, x3, x4 = thin(), thin(), thin(), thin()
                        E("joff").tensor_tensor(x1[:], arp, c_[:], OP.mult)
                        E("joff").tensor_tensor(x2[:], arq, s_[:], OP.mult)
                        E("joff").tensor_tensor(x3[:], arp, s_[:], OP.mult)
                        E("joff").tensor_tensor(x4[:], arq, c_[:], OP.mult)
                        E("joff").tensor_tensor(arp, x1[:], x2[:], OP.subtract)
                        E("joff").tensor_tensor(arq, x3[:], x4[:], OP.add)
                    elif prev_pq == rp_s:
                        # arp == 0: new arp = -arq*s ; new arq = arq*c
                        E("jthin").scalar_tensor_tensor(
                            arp, arq, -1.0, s_[:], OP.mult, OP.mult)
                        E("joff").tensor_tensor(arq, arq, c_[:], OP.mult)
                    else:
                        assert prev_pq == rq_s
                        # arq == 0: new arq = arp*s ; new arp = arp*c
                        E("joff").tensor_tensor(arq, arp, s_[:], OP.mult)
                        E("joff").tensor_tensor(arp, arp, c_[:], OP.mult)
                    prev_pq = (min(p_, q_), max(p_, q_))
                    cbf = tbf()
                    sbf = tbf()
                    E("csbf").activation(cbf[:], c_[:], AF.Copy)
                    E("csbf").activation(sbf[:], s_[:], AF.Copy)
                    if rn == 0:
                        # V = G1 directly (closed form, V was identity)
                        nc.gpsimd.memset(V[:], 0.0)
                        E("vinit").tensor_copy(_pl(V, (p_ * 3 + p_) * F, F),
                                               cbf[:])
                        E("vinit").tensor_scalar_mul(
                            _pl(V, (p_ * 3 + q_) * F, F), sbf[:], -1.0)
                        E("vinit").tensor_copy(_pl(V, (q_ * 3 + p_) * F, F),
                                               sbf[:])
                        E("vinit").tensor_copy(_pl(V, (q_ * 3 + q_) * F, F),
                                               cbf[:])
                        nc.gpsimd.memset(_pl(V, (r_ * 3 + r_) * F, F), 1.0)
                    else:
                        y1, y2, y3_, y4 = y3(), y3(), y3(), y3()
                        E("vupd").tensor_tensor(y1[:], VC(p_),
                                                _bc(cbf, 0, 3), OP.mult)
                        E("vupd").tensor_tensor(y2[:], VC(q_),
                                                _bc(sbf, 0, 3), OP.mult)
                        E("vupd").tensor_tensor(y3_[:], VC(p_),
                                                _bc(sbf, 0, 3), OP.mult)
                        E("vupd").tensor_tensor(y4[:], VC(q_),
                                                _bc(cbf, 0, 3), OP.mult)
                        E("vupd").tensor_tensor(VC(p_), y1[:], y2[:],
                                                OP.subtract)
                        E("vupd").tensor_tensor(VC(q_), y3_[:], y4[:],
                                                OP.add)

                # ---- sort + detV parity
                lam = [Ap(0, 0), Ap(1, 1), Ap(2, 2)]
                detV = thin()
                first = True
                for (i, j) in ((0, 1), (0, 2), (1, 2)):
                    m = thin()
                    E("srtthin").tensor_tensor(m[:], lam[j], lam[i], OP.is_gt)
                    lo = thin()
                    E("srtthin").tensor_tensor(lo[:], lam[i], lam[j], OP.min)
                    E("srtthin").tensor_tensor(lam[i], lam[i], lam[j], OP.max)
                    E("srtcopy").tensor_copy(lam[j], lo[:])
                    mbf = tbf()
                    E("csbf").activation(mbf[:], m[:], AF.Copy)
                    d3 = y3()
                    md = y3()
                    E("srtv").tensor_tensor(d3[:], VC(j), VC(i), OP.subtract)
                    E("srtv").tensor_tensor(md[:], d3[:], _bc(mbf, 0, 3),
                                            OP.mult)
                    E("srtv").tensor_tensor(VC(i), VC(i), md[:], OP.add)
                    E("srtv").tensor_tensor(VC(j), VC(j), md[:], OP.subtract)
                    if first:
                        nc.vector.tensor_scalar(detV[:], m[:], -2.0, 1.0,
                                                OP.mult, OP.add)
                        first = False
                    else:
                        f_ = thin()
                        nc.vector.tensor_scalar(f_[:], m[:], -2.0, 1.0,
                                                OP.mult, OP.add)
                        E("srtthin").tensor_tensor(detV[:], detV[:], f_[:],
                                                   OP.mult)

                # ---- random sign flips on V columns (one wide op)
                soff = 4 * F * k
                E("sgn").tensor_tensor(
                    V[:], V[:],
                    _ap(signs, soff, [[F, 3], [0, 3], [1, F]]), OP.mult)

                # ---- U
                U = grpp.tile([P, 9 * F], bf16, tag="U", name="U")

                def UC(i):
                    return _ap(U, i * 3 * F, [[F, 3], [1, F]])

                def Up(r, i):
                    return _pl(U, (i * 3 + r) * F, F)

                for i in range(2):
                    rl = thin()
                    E("rsact").activation(rl[:], lam[i], AF.Relu)
                    sg_ = thin()
                    E("rsact").activation(sg_[:], rl[:], AF.Sqrt)
                    sge = thin()
                    E("rsig").tensor_scalar_add(sge[:], sg_[:], 1e-20)
                    rs = thin()
                    nc.vector.reciprocal_approx_fast(rs[:], sge[:])
                    E("rsig").tensor_tensor(rs[:], rs[:], s_sc[:], OP.mult)
                    rsbf = tbf()
                    E("csbf").activation(rsbf[:], rs[:], AF.Copy)
                    wv = y3()
                    w2 = y3()
                    E("uassm").tensor_tensor(
                        wv[:], _pl(Hbf, 0, 3 * F),
                        _ap(V, (i * 3 + 0) * F, [[0, 3], [1, F]]), OP.mult)
                    E("uassm").tensor_tensor(
                        w2[:], _pl(Hbf, 3 * F, 3 * F),
                        _ap(V, (i * 3 + 1) * F, [[0, 3], [1, F]]), OP.mult)
                    E("uassm").tensor_tensor(wv[:], wv[:], w2[:], OP.add)
                    E("uassm").tensor_tensor(
                        w2[:], _pl(Hbf, 6 * F, 3 * F),
                        _ap(V, (i * 3 + 2) * F, [[0, 3], [1, F]]), OP.mult)
                    E("uassm").tensor_tensor(wv[:], wv[:], w2[:], OP.add)
                    E("uassm").tensor_tensor(UC(i), wv[:], _bc(rsbf, 0, 3),
                                             OP.mult)
                cr = [(1, 2), (2, 0), (0, 1)]
                for r in range(3):
                    a1, a2 = cr[r]
                    t1 = tbf()
                    t2 = tbf()
                    E("cross").tensor_tensor(t1[:], Up(a1, 0), Up(a2, 1),
                                             OP.mult)
                    E("cross").tensor_tensor(t2[:], Up(a2, 0), Up(a1, 1),
                                             OP.mult)
                    E("cross").tensor_tensor(Up(r, 2), t1[:], t2[:],
                                             OP.subtract)
                su2 = thin()
                E("su2").tensor_tensor(su2[:], detV[:],
                                       _pl(signs, soff + 3 * F, F), OP.mult)
                E("su2").tensor_tensor(su2[:], su2[:], inv_s[:], OP.mult)
                su2bf = tbf()
                E("csbf").activation(su2bf[:], su2[:], AF.Copy)
                E("su2").tensor_tensor(UC(2), UC(2), _bc(su2bf, 0, 3),
                                       OP.mult)

                # ---- R'' assembly (wide): R[a][b] = sum_i U[b,i] V[i,a]
                R = grpp.tile([P, 9 * F], bf16, tag="R", name="R")
                for a in range(3):
                    p1 = y3()
                    p2 = y3()
                    E("rassm").tensor_tensor(p1[:], UC(0), Vb(0, a), OP.mult)
                    E("rassm").tensor_tensor(p2[:], UC(1), Vb(1, a), OP.mult)
                    E("rassm").tensor_tensor(p1[:], p1[:], p2[:], OP.add)
                    E("rassm").tensor_tensor(p2[:], UC(2), Vb(2, a), OP.mult)
                    E("rassm").tensor_tensor(
                        _ap(R, a * 3 * F, [[F, 3], [1, F]]),
                        p1[:], p2[:], OP.add)
                if "nodist" in ablate:
                    dj = thin()
                    E("sthin").tensor_tensor(dj[:], _pl(R, 0, F), detV[:],
                                             OP.add)
                    E("dacc").tensor_tensor(acc[:], acc[:], dj[:], OP.add)
                    return
                dist_phase(k, PRB, R)

            def dist_phase(k, PRB, R):
                def Rb(a, b):
                    return _bc(R, (a * 3 + b) * F, J)

                d2t = qp.tile([P, JF], bf16, tag="q", name="d2t")
                for c in range(3):
                    q = qp.tile([P, JF], bf16, tag="q", name="q")
                    t2_ = qp.tile([P, JF], bf16, tag="q", name="q")
                    E("dmul").tensor_tensor(_rows(q, 0, J), _rows(PRB, 0, J),
                                            Rb(c, 0), OP.mult)
                    E("dmul").tensor_tensor(_rows(t2_, 0, J),
                                            _rows(PRB, JF, J), Rb(c, 1),
                                            OP.mult)
                    E("dadd").tensor_tensor(_rows(q, 0, J), _rows(q, 0, J),
                                            _rows(t2_, 0, J), OP.add)
                    E("dmul").tensor_tensor(_rows(t2_, 0, J),
                                            _rows(PRB, 2 * JF, J), Rb(c, 2),
                                            OP.mult)
                    E("dadd").tensor_tensor(_rows(q, 0, J), _rows(q, 0, J),
                                            _rows(t2_, 0, J), OP.add)
                    E("dadd").tensor_tensor(_rows(q, 0, J), _rows(q, 0, J),
                                            _rows(PRB, (3 + c) * JF, J),
                                            OP.subtract)
                    if c == 0:
                        E("dsq").activation(_rows(d2t, 0, J), _rows(q, 0, J),
                                            AF.Square)
                    else:
                        E("dsq").activation(_rows(q, 0, J), _rows(q, 0, J),
                                            AF.Square)
                        E("dcsum").tensor_tensor(_rows(d2t, 0, J),
                                                 _rows(d2t, 0, J),
                                                 _rows(q, 0, J), OP.add)
                E("dsqrt").activation(_rows(d2t, 0, J), _rows(d2t, 0, J),
                                      AF.Sqrt)
                T8 = trp.tile([P, 8 * F], bf16, tag="tr8", name="tr8")
                E("djtree").tensor_tensor(
                    _rows(T8, 0, 8), _rows(d2t, 0, 8),
                    _rows(d2t, 8 * F, 8), OP.add)
                for hw in (4, 2):
                    E("djtree").tensor_tensor(
                        _rows(T8, 0, hw), _rows(T8, 0, hw),
                        _rows(T8, hw * F, hw), OP.add)
                E("djtree").tensor_tensor(
                    _pl(T8, 0, F), _pl(T8, 0, F), _pl(T8, F, F), OP.add)
                dj = thin()
                E("djtree").tensor_tensor(dj[:], _pl(T8, 0, F),
                                          _pl(d2t, 16 * F, F), OP.add)
                E("dacc").tensor_tensor(acc[:], acc[:], dj[:], OP.add)

            def whole_body():
                sts = [stage1(k) for k in range(NCHUNK)]
                for k in range(NCHUNK):
                    stage2(k, sts[k])

            if iters == 1:
                whole_body()
            else:
                with tc.For_i(0, iters, 1):
                    whole_body()

            accs = persist.tile([P, 1], f32, tag="accs", name="accs")
            nc.vector.tensor_reduce(accs[:], acc[:], axis=AX.X, op=OP.add)
            nc.sync.dma_start(out_d[:], accs[:])

    nc.compile()
    return nc


_sign_planes = None


def sign_planes():
    global _sign_planes
    if _sign_planes is None:
        rng = np.random.default_rng(20260805)
        s = rng.choice(np.float32([-1.0, 1.0]), size=(3, P, F * NCHUNK))
        sp = np.empty((P, 4 * F * NCHUNK), np.float32)
        for k in range(NCHUNK):
            o = 4 * F * k
            for i in range(3):
                sp[:, o + i * F:o + (i + 1) * F] = s[i, :, k * F:(k + 1) * F]
            sp[:, o + 3 * F:o + 4 * F] = \
                (s[0] * s[1] * s[2])[:, k * F:(k + 1) * F]
        _sign_planes = sp.astype(ml_dtypes.bfloat16)
    return _sign_planes


_nc_cache = None


def get_nc():
    global _nc_cache
    if _nc_cache is None:
        _nc_cache = build_nc()
    return _nc_cache


def run(nc, pred, target, trace=False, **kw):
    pred2 = np.ascontiguousarray(np.asarray(pred), np.float32).reshape(B, JC)
    targ2 = np.ascontiguousarray(np.asarray(target), np.float32).reshape(B, JC)
    sp = sign_planes()
    in_maps = [
        {"pred": pred2[c * BC:(c + 1) * BC],
         "target": targ2[c * BC:(c + 1) * BC], "signs": sp}
        for c in range(NCORES)
    ]
    res = run_bass_kernel_spmd(nc, in_maps, list(range(NCORES)), trace=trace,
                               **kw)
    total = sum(r["partial"].astype(np.float64).sum() for r in res.results)
    loss = np.float32(total / (B * J))
    return loss, res


def kernel(pred, target):
    loss, _ = run(get_nc(), pred, target)
    return loss
